# revision 6
# baseline (speedup 1.0000x reference)
"""Trainium2 Bass kernel for nn_JointOrchestrator (ragged_sequence).

Reference computation:
  tok_n      = l2norm(tok_embs, axis=1)                       [T, H]
  gnn_sent_n = l2norm(gnn_sent_embs, axis=1)                  [N_sent, G]
  sent_out   = concat([tok_n, gnn_sent_n[tok_sent_ids]], -1)  [T, H+G]
  txt_prompt = mean(prompt_tokens, 0); gnn_prompt = W @ txt_prompt + b
  prompt_out = concat([l2norm(prompt_tokens,1), bcast(l2norm(gnn_prompt))], -1)
  out        = concat([sent_out, prompt_out], 0)              [T+P, H+G]

Sharding: data-parallel over the flat token dim T across 8 cores
(16384 tokens each); gnn_sent_embs / prompt / W / b replicated.
The gather uses indirect DMA from the raw DRAM table; gathered rows are
L2-normalized on-chip (normalize-then-gather == gather-then-normalize).
"""

import numpy as np

T, H, G, N_SENT, P = 131072, 768, 512, 4096, 8
N_CORES = 8
TS = T // N_CORES          # 16384 tokens per core
C = 4                      # 128-token chunks per block
NB = TS // (128 * C)       # 32 blocks
EPS = 1e-12

_CACHE = {}


def _build(repeat=1, variant=''):
    import concourse.bacc as bacc
    import concourse.mybir as mybir
    import concourse.tile as tile
    from concourse.bass import IndirectOffsetOnAxis
    from concourse.masks import make_identity

    f32 = mybir.dt.float32
    i32 = mybir.dt.int32
    AF = mybir.ActivationFunctionType
    X = mybir.AxisListType.X

    nc = bacc.Bacc("TRN2", target_bir_lowering=False, debug=False)

    tok = nc.dram_tensor("tok", [TS, H], f32, kind="ExternalInput")
    ids = nc.dram_tensor("ids", [128, NB * C], i32, kind="ExternalInput")
    sent = nc.dram_tensor("sent", [N_SENT, G], f32, kind="ExternalInput")
    prompt = nc.dram_tensor("prompt", [P, H], f32, kind="ExternalInput")
    w_t = nc.dram_tensor("w", [G, H], f32, kind="ExternalInput")
    b_t = nc.dram_tensor("b", [1, G], f32, kind="ExternalInput")
    out = nc.dram_tensor("out", [TS, H + G], f32, kind="ExternalOutput")
    pout = nc.dram_tensor("pout", [P, H + G], f32, kind="ExternalOutput")

    KH = H // 128  # 6 contraction chunks
    MG = G // 128  # 4 row chunks of W

    with tile.TileContext(nc) as tc:
        with (
            tc.tile_pool(name="const", bufs=1) as constp,
            tc.tile_pool(name="setup", bufs=1) as setupp,
            tc.tile_pool(name="psum", bufs=2, space="PSUM") as psump,
            tc.tile_pool(name="tin", bufs=3) as tinp,
            tc.tile_pool(name="tout", bufs=3) as toutp,
            tc.tile_pool(name="scr", bufs=2) as scrp,
            tc.tile_pool(name="stat", bufs=6) as statp,
        ):
            ident = constp.tile([128, 128], f32)
            make_identity(nc, ident[:])

            # ids come in pre-transposed: ids[p, t] = sent id of token t*128+p
            idsT = constp.tile([128, NB * C], i32)
            nc.sync.dma_start(idsT[:], ids[:])

            # ---- prompt branch (tiny) ----------------------------------
            p_sb = setupp.tile([P, H], f32)
            nc.sync.dma_start(p_sb[:], prompt[:])
            w_sb = setupp.tile([128, MG, H], f32)
            nc.sync.dma_start(
                w_sb[:], w_t[:].rearrange("(m p) h -> p m h", p=128)
            )
            b_sb = setupp.tile([1, G], f32)
            nc.sync.dma_start(b_sb[:], b_t[:])

            # txt_sum[k*128+j] = sum_i prompt[i, k*128+j]  (partition layout)
            txt_sb = setupp.tile([128, KH], f32)
            for k in range(KH):
                pt_ps = psump.tile([128, P], f32, tag="pt_ps")
                nc.tensor.transpose(
                    pt_ps[:], p_sb[:, k * 128:(k + 1) * 128], ident[:P, :P]
                )
                nc.vector.reduce_sum(txt_sb[:, k:k + 1], pt_ps[:], axis=X)

            # W^T chunks then g_psum[1, G] = sum_k txt_k^T @ (W^T)_k
            g_ps = psump.tile([1, G], f32)
            for k in range(KH):
                wt_ps = psump.tile([128, G], f32, tag="wt_ps")
                for m in range(MG):
                    nc.tensor.transpose(
                        wt_ps[:, m * 128:(m + 1) * 128],
                        w_sb[:, m, k * 128:(k + 1) * 128],
                        ident[:],
                    )
                wt_sb = setupp.tile([128, G], f32, tag="wt_sb")
                nc.vector.tensor_copy(wt_sb[:], wt_ps[:])
                nc.tensor.matmul(
                    g_ps[:], lhsT=txt_sb[:, k:k + 1], rhs=wt_sb[:],
                    start=(k == 0), stop=(k == KH - 1),
                )

            # gnn_prompt = g_psum/P + b, then l2norm
            g_sb = setupp.tile([1, G], f32)
            nc.vector.tensor_scalar_mul(g_sb[:], g_ps[:], 1.0 / P)
            nc.vector.tensor_add(g_sb[:], g_sb[:], b_sb[:])
            g_sq = setupp.tile([1, G], f32)
            g_ss = setupp.tile([1, 1], f32)
            nc.scalar.activation(g_sq[:], g_sb[:], AF.Square, accum_out=g_ss[:])
            nc.scalar.sqrt(g_ss[:], g_ss[:])
            nc.vector.tensor_scalar_max(g_ss[:], g_ss[:], EPS)
            g_inv = setupp.tile([1, 1], f32)
            nc.vector.reciprocal(g_inv[:], g_ss[:])
            g_n = setupp.tile([1, G], f32)
            nc.scalar.mul(g_n[:], g_sb[:], g_inv[:])

            # prompt token rows: l2norm then concat broadcast gnn_prompt_n
            po_sb = setupp.tile([P, H + G], f32)
            p_sq = setupp.tile([P, H], f32)
            p_ss = setupp.tile([P, 1], f32)
            nc.scalar.activation(p_sq[:], p_sb[:], AF.Square, accum_out=p_ss[:])
            nc.scalar.sqrt(p_ss[:], p_ss[:])
            nc.vector.tensor_scalar_max(p_ss[:], p_ss[:], EPS)
            p_inv = setupp.tile([P, 1], f32)
            nc.vector.reciprocal(p_inv[:], p_ss[:])
            nc.scalar.mul(po_sb[:, :H], p_sb[:], p_inv[:])
            ones8 = constp.tile([1, P], f32)
            nc.vector.memset(ones8[:], 1.0)
            bc_ps = psump.tile([P, G], f32)
            nc.tensor.matmul(bc_ps[:], lhsT=ones8[:], rhs=g_n[:],
                             start=True, stop=True)
            nc.vector.tensor_copy(po_sb[:, H:], bc_ps[:])
            nc.sync.dma_start(pout[:], po_sb[:])

            # ---- main token loop ---------------------------------------
            tok_v = tok[:].rearrange("(n c p) h -> n p c h", c=C, p=128)
            out_v = out[:].rearrange("(n c p) h -> n p c h", c=C, p=128)
            for nb in [i for _ in range(repeat) for i in range(NB)]:
                tin = tinp.tile([128, C, H], f32)
                if variant != 'noload':
                    nc.sync.dma_start(tin[:], tok_v[nb])
                tout_t = toutp.tile([128, C, H + G], f32)

                # token-side L2 norm on ACT (square+accum, sqrt, scaled copy)
                ss = statp.tile([128, C], f32, tag="ss")
                inv = statp.tile([128, C], f32, tag="inv")
                sqd = scrp.tile([128, H], f32, tag="sqd")
                for c in range(C):
                    nc.scalar.activation(sqd[:], tin[:, c, :], AF.Square,
                                         accum_out=ss[:, c:c + 1])
                nc.scalar.sqrt(ss[:], ss[:])
                nc.vector.tensor_scalar_max(ss[:], ss[:], EPS)
                nc.vector.reciprocal(inv[:], ss[:])
                for c in range(C):
                    nc.scalar.mul(tout_t[:, c, :H], tin[:, c, :], inv[:, c:c + 1])

                # gather raw sentence rows, then normalize in place (DVE)
                for c in range(C if variant != 'nogather' else 0):
                    nc.gpsimd.indirect_dma_start(
                        out=tout_t[:, c, H:],
                        out_offset=None,
                        in_=sent[:],
                        in_offset=IndirectOffsetOnAxis(
                            ap=idsT[:, nb * C + c: nb * C + c + 1], axis=0
                        ),
                    )
                gsq = scrp.tile([128, C, G], f32, tag="gsq")
                gss = statp.tile([128, C], f32, tag="gss")
                ginv = statp.tile([128, C], f32, tag="ginv")
                if variant == 'nogather':
                    nc.vector.memset(tout_t[:, :, H:], 0.125)
                nc.vector.tensor_tensor(
                    out=gsq[:], in0=tout_t[:, :, H:], in1=tout_t[:, :, H:],
                    op=mybir.AluOpType.mult,
                )
                nc.vector.reduce_sum(gss[:], gsq[:], axis=X)
                nc.scalar.sqrt(gss[:], gss[:])
                nc.vector.tensor_scalar_max(gss[:], gss[:], EPS)
                nc.vector.reciprocal(ginv[:], gss[:])
                for c in range(C):
                    nc.vector.tensor_scalar_mul(
                        tout_t[:, c, H:], tout_t[:, c, H:], ginv[:, c:c + 1]
                    )

                # store on the Act HWDGE ring so it doesn't serialize with
                # the loads on the SP ring (FIFO per ring)
                if variant != 'nostore':
                    nc.scalar.dma_start(out_v[nb], tout_t[:])
                else:
                    nc.scalar.dma_start(out_v[nb][:, :1, :1], tout_t[:, :1, :1])

    nc.compile()
    return nc


def _get_nc(repeat=1, variant=''):
    key = ("nc", repeat, variant)
    if key not in _CACHE:
        _CACHE[key] = _build(repeat, variant)
    return _CACHE[key]


def make_in_maps(tok_embs, gnn_sent_embs, prompt_tokens, W, b, tok_sent_ids):
    tok_embs = np.ascontiguousarray(np.asarray(tok_embs, dtype=np.float32))
    sent = np.ascontiguousarray(np.asarray(gnn_sent_embs, dtype=np.float32))
    prompt = np.ascontiguousarray(np.asarray(prompt_tokens, dtype=np.float32))
    W = np.ascontiguousarray(np.asarray(W, dtype=np.float32))
    b = np.ascontiguousarray(np.asarray(b, dtype=np.float32)).reshape(1, G)
    ids = np.asarray(tok_sent_ids).astype(np.int32)
    in_maps = []
    for cid in range(N_CORES):
        sl = slice(cid * TS, (cid + 1) * TS)
        in_maps.append({
            "tok": np.ascontiguousarray(tok_embs[sl]),
            "ids": np.ascontiguousarray(ids[sl].reshape(NB * C, 128).T),
            "sent": sent,
            "prompt": prompt,
            "w": W,
            "b": b,
        })
    return in_maps


def run(inputs, trace=False, **kwargs):
    from concourse.bass_utils import run_bass_kernel_spmd

    nc = _get_nc()
    in_maps = make_in_maps(**inputs)
    res = run_bass_kernel_spmd(
        nc, in_maps, core_ids=list(range(N_CORES)), trace=trace, **kwargs
    )
    full = np.empty((T + P, H + G), dtype=np.float32)
    for cid in range(N_CORES):
        full[cid * TS:(cid + 1) * TS] = res.results[cid]["out"]
    full[T:] = res.results[0]["pout"]
    return full, res


def kernel(**inputs) -> np.ndarray:
    full, _ = run(inputs, trace=False)
    return full
